# revision 18
# baseline (speedup 1.0000x reference)
"""DeepTopK (topk_masking) Trainium2 kernel — 8 NeuronCores, data-parallel over tokens.

Math per reference: 3 fused linear+relu+global-topk-mask layers + final linear.
  h1 = topk_mask(relu(x @ W1 + b1), 64*4096)      [4096, 4096]
  h2 = topk_mask(relu(h1 @ W2 + b2), 128*4096)    [4096, 16384]
  h3 = topk_mask(relu(h2 @ Wd2 + bd2), 64*4096)   [4096, 4096]
  out = h3 @ Wd1 + bd1                            [4096, 1024]

Design notes (hardware + numpy-sim measured):
- top-k masks amplify value noise via borderline rank flips (~sqrt(eps)):
  layers feeding masks 1/2 need the 3-term hi/lo f32r split (~1e-7 rel);
  mask 3 tolerates 1-pass f32r (13-bit) noise, and the final linear is
  insensitive — so passes=(3,3,1,1). 1-pass matmuls stream f32 weights
  straight from HBM bitcast to f32r (no split/copy engine work).
- Data-parallel over tokens: each core owns 512 tokens, streams ALL weights
  from HBM (hidden under PE time). Activations feature-major [feat, tok].
  h1/h3 stay SBUF-resident (f32) and are masked in place; only h2 round-trips
  through DRAM (too big for SBUF), masked on reload during L3.
- Global top-k threshold: count-multisection over a per-core top-8-per-block
  summary (exact counts at rank ~m), with a few small collectives per mask.
"""
import sys
import numpy as np

for _p in ("/opt/trn_rl_repo",):
    if _p not in sys.path:
        sys.path.insert(0, _p)

import concourse.bass as bass
import concourse.bacc as bacc
import concourse.mybir as mybir
import concourse.tile as tile
from concourse.bass_utils import run_bass_kernel_spmd


def _ensure_profile_hook():
    """bass_utils trace=True under axon imports antenv.axon_hooks, which this
    image lacks; provide it so NTFF profiling works (no-op if already there)."""
    import types
    try:
        import antenv.axon_hooks  # noqa: F401
        return
    except ImportError:
        pass
    mod = types.ModuleType("antenv.axon_hooks")
    _state = {"hook": None}

    def set_axon_ntff_profile_hook(hook):
        _state["hook"] = hook

    def get_axon_ntff_profile_hook():
        if _state["hook"] is None:
            try:
                from trn_agent_boot.trn_boot import _ntff_profile_via_ctypes
                _state["hook"] = _ntff_profile_via_ctypes("/opt/axon/libaxon_pjrt.so")
            except Exception:
                _state["hook"] = None
        return _state["hook"]

    mod.set_axon_ntff_profile_hook = set_axon_ntff_profile_hook
    mod.get_axon_ntff_profile_hook = get_axon_ntff_profile_hook
    sys.modules["antenv.axon_hooks"] = mod
    try:
        import antenv
        antenv.axon_hooks = mod
    except ImportError:
        pass


_ensure_profile_hook()
LAST_EXEC_NS = None
LAST_DBG = None

F32 = mybir.dt.float32
F32R = mybir.dt.float32r
ALU = mybir.AluOpType
AFT = mybir.ActivationFunctionType
AX = mybir.AxisListType

FULL_CFG = dict(
    n_cores=8,
    d_model=1024,
    d_mid=4096,
    d_feat=16384,
    n_tok=4096,
    k_mid=64,
    k_feat=128,
    blks=(32, 64, 32),        # exact summary block length per mask (tokens)
    blks_coarse=(128, 256, 128),  # coarse summary blocks (4x exact; derived)
    rounds_coarse=2,
    rounds_band=4,
    G=15,                     # grid points per round
    # upper bounds per mask: ~3-7x above the data-distribution thresholds
    # (randn x, kaiming weights -> t ~= (1.25, 0.26, 0.035)); tighter bounds
    # let 2 coarse rounds narrow enough for the exact band phase.
    hi0=(4.0, 1.0, 0.25),
    passes=(3, 3, 1, 1),      # matmul passes per layer (3 = hi/lo split)
)


def _ceil_div(a, b):
    return (a + b - 1) // b


class _LayerCtx:
    """Holds the pools shared by all layers."""

    def __init__(self, nc, tc, ctx, cfg):
        self.nc, self.tc, self.cfg = nc, tc, cfg
        p = lambda name, bufs, space="SBUF": ctx.enter_context(
            tc.tile_pool(name=name, bufs=bufs, space=space)
        )
        self.persist = p("persist", 1)
        self.hb = p("hb", 1)
        self.wf = p("wf", 6)
        self.wsh = p("wsh", 4)
        self.wsl = p("wsl", 2)
        self.rhsd = p("rhsd", 5)
        self.rhsr = p("rhsr", 4)
        self.rhs = p("rhs", 2)
        self.ev = p("ev", 2)
        self.psum = p("psum", 1, "PSUM")
        self.dram = p("dram", 1, "DRAM")
        self.thr = p("thr", 1)


def emit_layer(
    L, name, w_dram, bias_dram, K, M, N,
    rhs_src,          # list of SBUF f32 tiles (len K/128, premasked) or DRAM [K, N]
    mask_t,           # [128,1] threshold AP applied on DRAM rhs load, or None
    relu,             # bool
    out_dst,          # "sbuf" -> returns list of f32 tiles; or DRAM tensor [M, N]
    s1_exact,         # (tile, blk) or None: exact top-8-per-blk summary sink
    s1_coarse,        # (tile, 2*blk) or None: coarse summary (from exact)
    passes=1,
    m_block=8,
):
    nc, cfg = L.nc, L.cfg
    kc = K // 128
    mc = M // 128
    m_block = min(m_block, mc)
    nq = _ceil_div(mc, m_block)
    assert mc % m_block == 0 and m_block % 2 == 0

    # bias: [M,1] dram -> [128, mc] sbuf (column m = bias slice of M-tile m)
    bias_sb = L.persist.tile([128, mc], F32, name=f"{name}_bias", tag=f"{name}_bias")
    nc.sync.dma_start(bias_sb[:], bias_dram.ap().rearrange("(a p) o -> p (a o)", p=128))

    out_tiles = []
    for q in range(nq):
        mlo = q * m_block
        mhi = min(mc, mlo + m_block)
        nm = mhi - mlo
        ps = [L.psum.tile([128, N], F32, name=f"ps{i}", tag=f"ps{i}") for i in range(nm)]
        for k in range(kc):
            # --- rhs chunk ---
            if isinstance(rhs_src, list):
                rf = rhs_src[k][:]
            else:
                rt = L.rhsd.tile([128, N], F32, name="rh_dma", tag="rh_dma")
                nc.sync.dma_start(rt[:], rhs_src[k * 128:(k + 1) * 128, :])
                rf = rt[:]
            if passes == 3:
                if mask_t is not None:
                    rm = L.rhs.tile([128, N], F32, name="rh_m", tag="rh_m")
                    nc.vector.scalar_tensor_tensor(
                        rm[:], rf, mask_t, rf, op0=ALU.is_ge, op1=ALU.mult)
                    rf = rm[:]
                rh = L.rhs.tile([128, N], F32R, name="rh_h", tag="rh_h")
                rl = L.rhs.tile([128, N], F32R, name="rh_l", tag="rh_l")
                nc.scalar.copy(rh[:], rf)
                nc.vector.tensor_tensor(rl[:], rf, rh[:].bitcast(F32), op=ALU.subtract)
            else:
                # fp32r operands must be produced rounded; fuse mask + round
                rr = L.rhsr.tile([128, N], F32R, name="rh_r", tag="rh_r")
                if mask_t is not None:
                    nc.vector.scalar_tensor_tensor(
                        rr[:], rf, mask_t, rf, op0=ALU.is_ge, op1=ALU.mult)
                else:
                    nc.scalar.copy(rr[:], rf)

            st = (k == 0)
            sp = (k == kc - 1)
            # --- weight panel in halves of [128, 512] for pipeline depth ---
            mh = m_block // 2
            for h in range(2):
                cl = (mlo + h * mh) * 128
                wf = L.wf.tile([128, mh * 128], F32, name="wf", tag="wf")
                # scalar-queue HWDGE: keeps weight DMA issue off the busy
                # sync queue (3 DMAs/chunk serialized there gate L3's cadence)
                nc.scalar.dma_start(
                    wf[:], w_dram[k * 128:(k + 1) * 128, cl:cl + mh * 128])
                wh = L.wsh.tile([128, mh * 128], F32R, name="wh", tag="wh")
                nc.scalar.copy(wh[:], wf[:])
                if passes == 3:
                    wl = L.wsl.tile([128, mh * 128], F32R, name="wl", tag="wl")
                    nc.vector.tensor_tensor(
                        wl[:], wf[:], wh[:].bitcast(F32), op=ALU.subtract)
                for mj in range(mh):
                    mi = h * mh + mj
                    wha = wh[:, mj * 128:(mj + 1) * 128]
                    if passes == 3:
                        wla = wl[:, mj * 128:(mj + 1) * 128]
                        nc.tensor.matmul(ps[mi][:], wha, rh[:], start=st, stop=False)
                        nc.tensor.matmul(ps[mi][:], wha, rl[:], start=False, stop=False)
                        nc.tensor.matmul(ps[mi][:], wla, rh[:], start=False, stop=sp)
                    else:
                        nc.tensor.matmul(ps[mi][:], wha, rr[:], start=st, stop=sp)

        # --- evacuate + bias + (relu) + summary + sink ---
        for mi in range(nm):
            mg = mlo + mi
            if out_dst == "sbuf":
                ot = L.hb.tile([128, N], F32, name=f"hbuf{mg}", tag=f"hbuf{mg}")
            else:
                ot = L.ev.tile([128, N], F32, name="ev", tag="ev")
            nc.scalar.activation(
                ot[:], ps[mi][:], AFT.Relu if relu else AFT.Identity,
                bias=bias_sb[:, mg:mg + 1], scale=1.0)
            if s1_exact is not None:
                s1t, blk = s1_exact
                nblk = N // blk
                base = mg * nblk * 8
                for c in range(nblk):
                    nc.vector.max(
                        s1t[:, base + c * 8: base + c * 8 + 8],
                        ot[:, c * blk:(c + 1) * blk])
                if s1_coarse is not None:
                    s2t, cblk = s1_coarse
                    nc2 = N // cblk
                    base2 = mg * nc2 * 8
                    rw = (cblk // blk) * 8  # exact slots per coarse block
                    for c in range(nc2):
                        nc.vector.max(
                            s2t[:, base2 + c * 8: base2 + c * 8 + 8],
                            s1t[:, base + c * rw: base + c * rw + rw])
            if out_dst == "sbuf":
                out_tiles.append(ot)
            else:
                nc.sync.dma_start(out_dst[mg * 128:(mg + 1) * 128, :], ot[:])
    return out_tiles


def mask_inplace(L, tiles, t_ap):
    """tiles <- tiles * (tiles >= t) elementwise, in place."""
    nc = L.nc
    for t in tiles:
        nc.vector.scalar_tensor_tensor(
            t[:], t[:], t_ap, t[:], op0=ALU.is_ge, op1=ALU.mult)


def _count_round(L, name, r, data_ap, S, grid, cnts, scratch, split=None):
    """15 count passes of data >= grid_g, accum per partition into cnts.
    split=(gridneg, scratch2): run grid points 8..G-1 on ACT via
    sign(x - t) accumulation (count = (acc + S)/2); exact when no data
    value equals t (coarse rounds tolerate the 0.5-count tie case)."""
    nc = L.nc
    G = L.cfg["G"]
    nc.vector.memset(cnts[:], 0.0)
    ndve = G if split is None else 8
    for g in range(ndve):
        nc.vector.tensor_scalar(
            scratch[:, :S], data_ap, grid[:, g:g + 1], 0.0,
            op0=ALU.is_ge, op1=ALU.add, accum_out=cnts[:, g:g + 1])
    if split is not None:
        gridneg, scratch2 = split
        nc.vector.tensor_scalar(gridneg[:], grid[:], -1.0, None, op0=ALU.mult)
        for g in range(8, G):
            nc.scalar.activation(
                scratch2[:, :S], data_ap, AFT.Sign,
                bias=gridneg[:, g:g + 1], scale=1.0,
                accum_out=cnts[:, g:g + 1])
        # count = (acc + S) / 2 for the ACT columns
        nc.vector.tensor_scalar(
            cnts[:, 8:G], cnts[:, 8:G], float(S), 0.5,
            op0=ALU.add, op1=ALU.mult)


def emit_threshold_v2(L, name, s1x, Sx, s1c, Sc, m_count, hi0, iota_f, scratch,
                      n_cores):
    """Coarse rounds on half-size summary, then band-compact + cheap rounds.
    Exact final threshold (count == m) with ~8 small AllReduces."""
    nc, cfg = L.nc, L.cfg
    G = cfg["G"]
    RC, RB = cfg.get("rounds_coarse", 3), cfg.get("rounds_band", 5)
    UG = 64.0  # coarse-summary undercount guard for hi-updates
    P = L.thr
    T = lambda sh, tg: P.tile(sh, F32, name=f"{name}{tg}", tag=f"{name}{tg}")
    lo, hi = T([128, 1], "_lo"), T([128, 1], "_hi")
    nc.vector.memset(lo[:], 0.0)
    nc.vector.memset(hi[:], hi0)
    cnts, cntg = T([128, G + 1], "_cn"), T([128, G + 1], "_cg")
    grid, t15 = T([128, G], "_gr"), T([128, G], "_t15")
    d1, r1, c0 = T([128, 1], "_d1"), T([128, 1], "_r1"), T([128, 1], "_c0")
    band = P.tile([128, Sx // 8], F32, name="band", tag="band")
    zpc = P.tile([128, min(256, Sx)], F32, name="zpc", tag="zpc")
    cc_in = L.dram.tile([1, G + 1], F32, name=f"{name}_ci", tag=f"{name}_ci")
    cc_out = L.dram.tile([1, G + 1], F32, name=f"{name}_co", tag=f"{name}_co")

    def mkgrid():
        nc.vector.tensor_tensor(d1[:], hi[:], lo[:], op=ALU.subtract)
        nc.vector.tensor_scalar(d1[:], d1[:], 1.0 / (G + 1), None, op0=ALU.mult)
        nc.vector.tensor_scalar(grid[:], iota_f[:], d1[:], None, op0=ALU.mult)
        nc.vector.tensor_scalar(grid[:], grid[:], lo[:], None, op0=ALU.add)

    def allreduce(ncols):
        nc.gpsimd.partition_all_reduce(
            cntg[:, :ncols], cnts[:, :ncols], channels=128,
            reduce_op=bass.bass_isa.ReduceOp.add)
        if n_cores > 1:
            nc.sync.dma_start(cc_in[:, :ncols], cntg[0:1, :ncols])
            nc.gpsimd.collective_compute(
                "AllReduce", ALU.add,
                replica_groups=[list(range(n_cores))],
                ins=[cc_in[:, :ncols]], outs=[cc_out[:, :ncols]])
            nc.sync.dma_start(cntg[0:1, :ncols], cc_out[:, :ncols])
            nc.gpsimd.partition_broadcast(
                cntg[:, :ncols], cntg[:, :ncols], channels=128)

    def update(mval, guard):
        # lo' = max(lo, max(grid*[cnt>=m])); hi' = min(hi, min(grid + [cnt>=m-guard]*BIG))
        nc.vector.tensor_scalar(t15[:], cntg[:, :G], float(mval), None, op0=ALU.is_ge)
        nc.vector.tensor_tensor(t15[:], grid[:], t15[:], op=ALU.mult)
        nc.vector.tensor_reduce(r1[:], t15[:], axis=AX.X, op=ALU.max)
        nc.vector.tensor_tensor(lo[:], lo[:], r1[:], op=ALU.max)
        nc.vector.tensor_scalar(t15[:], cntg[:, :G], float(mval) - guard, None, op0=ALU.is_ge)
        nc.vector.tensor_scalar(t15[:], t15[:], 1e30, None, op0=ALU.mult)
        nc.vector.tensor_tensor(t15[:], grid[:], t15[:], op=ALU.add)
        nc.vector.tensor_reduce(r1[:], t15[:], axis=AX.X, op=ALU.min)
        nc.vector.tensor_tensor(hi[:], hi[:], r1[:], op=ALU.min)

    gridneg = T([128, G], "_grn")
    scratch2 = P.tile([128, Sc], mybir.dt.bfloat16,
                      name=f"{name}_sc2", tag="scratch2")
    for r in range(RC):
        mkgrid()
        _count_round(L, name, r, s1c[:, :Sc], Sc, grid, cnts, scratch,
                     split=(gridneg, scratch2))
        allreduce(G)
        update(m_count, UG)

    # band-compact the exact summary below hi; C0 = exact count(s1x >= hi)
    # (batch the is_lt masking in [128, 512] chunks to cut DVE dispatch count)
    ZW = min(256, Sx)
    for cb in range(Sx // ZW):
        nc.vector.scalar_tensor_tensor(
            zpc[:, :ZW], s1x[:, cb * ZW:(cb + 1) * ZW], hi[:],
            s1x[:, cb * ZW:(cb + 1) * ZW], op0=ALU.is_lt, op1=ALU.mult)
        for c in range(ZW // 64):
            g = cb * (ZW // 64) + c
            nc.vector.max(band[:, g * 8:(g + 1) * 8], zpc[:, c * 64:(c + 1) * 64])
    SB = Sx // 8

    # second-level compact to [128, W2] + piggyback local C0; AllGather once,
    # then the final rounds run replicated-locally (no more collectives).
    B2 = max(8, SB // 16)
    n2 = SB // B2
    W2 = n2 * 8
    band2 = P.tile([128, W2], F32, name="band2", tag="band2")
    for c in range(n2):
        nc.vector.max(band2[:, c * 8:(c + 1) * 8], band[:, c * B2:(c + 1) * B2])
    H = Sx // 2
    nc.vector.tensor_scalar(
        scratch[:, :H], s1x[:, :H], hi[:], 0.0,
        op0=ALU.is_ge, op1=ALU.add, accum_out=c0[:])
    nc.vector.tensor_scalar(
        scratch[:, :H], s1x[:, H:Sx], hi[:], 0.0,
        op0=ALU.is_ge, op1=ALU.add, accum_out=r1[:])
    nc.vector.tensor_tensor(c0[:], c0[:], r1[:], op=ALU.add)

    GW = n_cores * (W2 + 1)
    gsb = P.tile([128, GW], F32, name=f"{name}_gsb", tag="gsb")
    if n_cores > 1:
        agin = L.dram.tile([128, W2 + 1], F32, name=f"{name}_agi", tag=f"{name}_agi")
        agout = L.dram.tile([128, GW], F32, name=f"{name}_ago",
                            tag=f"{name}_ago", addr_space="Shared")
        nc.sync.dma_start(agin[:, :W2], band2[:])
        nc.sync.dma_start(agin[:, W2:W2 + 1], c0[:])
        nc.gpsimd.collective_compute(
            "AllGather", ALU.bypass,
            replica_groups=[list(range(n_cores))],
            ins=[agin[:]], outs=[agout[:]])
        nc.sync.dma_start(gsb[:], agout[:])
    else:
        nc.vector.tensor_copy(gsb[:, :W2], band2[:])
        nc.vector.tensor_copy(gsb[:, W2:W2 + 1], c0[:])
    # strided views over the gathered payload
    g3 = gsb[:].rearrange("p (r w) -> p r w", w=W2 + 1)
    gvals = g3[:, :, 0:W2]
    gc0 = g3[:, :, W2:W2 + 1]
    # global C0 (replicated): sum ranks' per-partition partials, then partitions
    nc.vector.tensor_reduce(c0[:], gc0, axis=AX.XY, op=ALU.add)
    nc.gpsimd.partition_all_reduce(
        c0[:], c0[:], channels=128, reduce_op=bass.bass_isa.ReduceOp.add)

    for r in range(RB):
        mkgrid()
        nc.vector.memset(cnts[:], 0.0)
        for g in range(G):
            nc.vector.tensor_scalar(
                scratch[:, :n_cores * W2], gvals, grid[:, g:g + 1], 0.0,
                op0=ALU.is_ge, op1=ALU.add, accum_out=cnts[:, g:g + 1])
        nc.gpsimd.partition_all_reduce(
            cntg[:, :G], cnts[:, :G], channels=128,
            reduce_op=bass.bass_isa.ReduceOp.add)
        nc.vector.tensor_scalar(
            cntg[:, :G], cntg[:, :G], c0[:], None, op0=ALU.add)
        update(m_count, 0.0)
    return lo


def build(cfg):
    from contextlib import ExitStack

    n_cores = cfg["n_cores"]
    DM, DMID, DF = cfg["d_model"], cfg["d_mid"], cfg["d_feat"]
    NTOK = cfg["n_tok"]
    N = NTOK // n_cores
    blk1, blk2, blk3 = cfg["blks"]
    m1 = cfg["k_mid"] * NTOK
    m2 = cfg["k_feat"] * NTOK
    m3 = cfg["k_mid"] * NTOK
    G = cfg["G"]
    p1, p2, p3, p4 = cfg["passes"]

    nc = bacc.Bacc("TRN2", target_bir_lowering=False, debug=False,
                   num_devices=n_cores)
    xT = nc.declare_dram_parameter("xT", [DM, N], F32, isOutput=False)
    W1 = nc.declare_dram_parameter("W1", [DM, DMID], F32, isOutput=False)
    b1 = nc.declare_dram_parameter("b1", [DMID, 1], F32, isOutput=False)
    W2 = nc.declare_dram_parameter("W2", [DMID, DF], F32, isOutput=False)
    b2 = nc.declare_dram_parameter("b2", [DF, 1], F32, isOutput=False)
    Wd2 = nc.declare_dram_parameter("Wd2", [DF, DMID], F32, isOutput=False)
    bd2 = nc.declare_dram_parameter("bd2", [DMID, 1], F32, isOutput=False)
    Wd1 = nc.declare_dram_parameter("Wd1", [DMID, DM], F32, isOutput=False)
    bd1 = nc.declare_dram_parameter("bd1", [DM, 1], F32, isOutput=False)
    recon = nc.declare_dram_parameter("recon", [DM, N], F32, isOutput=True)
    dbg = nc.declare_dram_parameter("dbg", [1, 4], F32, isOutput=True)

    h2buf = nc.dram_tensor("h2buf", [DF, N], F32)

    S1 = DMID * (N // blk1) * 8 // 128   # summary width per partition, mask1/3
    S2 = DF * (N // blk2) * 8 // 128
    S3 = DMID * (N // blk3) * 8 // 128
    c1, c2, c3 = cfg["blks_coarse"]
    C1 = DMID * (N // c1) * 8 // 128
    C2 = DF * (N // c2) * 8 // 128
    C3 = DMID * (N // c3) * 8 // 128

    with ExitStack() as ctx:
        tc = ctx.enter_context(tile.TileContext(nc))
        L = _LayerCtx(nc, tc, ctx, cfg)

        iota_i = L.thr.tile([128, G], mybir.dt.int32, name="iota_i", tag="iota_i")
        nc.gpsimd.iota(iota_i[:], pattern=[[1, G]], base=1, channel_multiplier=0)
        iota_f = L.thr.tile([128, G], F32, name="iota_f", tag="iota_f")
        nc.vector.tensor_copy(iota_f[:], iota_i[:])
        _sw = max(max(S1, S2, S3) // 2, n_cores * 128)
        scratch = L.thr.tile([128, _sw], mybir.dt.bfloat16,
                             name="scratch", tag="scratch")

        s1a = L.persist.tile([128, S1], F32, name="s1a", tag="s1a")
        s1a2 = L.persist.tile([128, C1], F32, name="s1a2", tag="s1a2")
        h1 = emit_layer(L, "L1", W1, b1, DM, DMID, N, xT, None, True,
                        "sbuf", (s1a, blk1), (s1a2, c1), passes=p1)
        t1 = emit_threshold_v2(L, "t1", s1a, S1, s1a2, C1, m1, cfg["hi0"][0],
                               iota_f, scratch, n_cores)
        mask_inplace(L, h1, t1[:])

        s1b = L.persist.tile([128, S2], F32, name="s1b", tag="s1b")
        s1b2 = L.persist.tile([128, C2], F32, name="s1b2", tag="s1b2")
        emit_layer(L, "L2", W2, b2, DMID, DF, N, h1, None, True,
                   h2buf, (s1b, blk2), (s1b2, c2), passes=p2)
        t2 = emit_threshold_v2(L, "t2", s1b, S2, s1b2, C2, m2, cfg["hi0"][1],
                               iota_f, scratch, n_cores)

        s1c = L.persist.tile([128, S3], F32, name="s1c", tag="s1a")
        s1c2 = L.persist.tile([128, C3], F32, name="s1c2", tag="s1a2")
        h3 = emit_layer(L, "L3", Wd2, bd2, DF, DMID, N, h2buf, t2[:], True,
                        "sbuf", (s1c, blk3), (s1c2, c3), passes=p3)
        t3 = emit_threshold_v2(L, "t3", s1c, S3, s1c2, C3, m3, cfg["hi0"][2],
                               iota_f, scratch, n_cores)
        # h3 mask fuses into L4's per-chunk f32r rounding (mask_t on list rhs)
        emit_layer(L, "L4", Wd1, bd1, DMID, DM, N, h3, t3[:], False,
                   recon, None, None, passes=p4)

        tdbg = L.thr.tile([128, 4], F32, name="tdbg", tag="tdbg")
        nc.vector.memset(tdbg[:], 0.0)
        nc.vector.tensor_copy(tdbg[:, 0:1], t1[:])
        nc.vector.tensor_copy(tdbg[:, 1:2], t2[:])
        nc.vector.tensor_copy(tdbg[:, 2:3], t3[:])
        nc.sync.dma_start(dbg[:], tdbg[0:1, :])

    nc.compile()
    return nc


_CACHE = {}


def _get_nc(cfg):
    key = tuple(sorted((k, v if not isinstance(v, tuple) else v) for k, v in cfg.items()))
    if key not in _CACHE:
        _CACHE[key] = build(cfg)
    return _CACHE[key]


def kernel(x, W_enc1, b_enc1, W_enc2, b_enc2, W_dec2, b_dec2, W_dec1, b_dec1,
           k_mid, k_feat, _cfg=None):
    cfg = dict(_cfg or FULL_CFG)
    cfg["k_mid"] = int(k_mid)
    cfg["k_feat"] = int(k_feat)
    n_cores = cfg["n_cores"]
    N = cfg["n_tok"] // n_cores

    nc = _get_nc(cfg)

    f32 = lambda a: np.ascontiguousarray(np.asarray(a), dtype=np.float32)
    xT = f32(x).T.copy()
    com = dict(
        W1=f32(W_enc1), b1=f32(b_enc1).reshape(-1, 1),
        W2=f32(W_enc2), b2=f32(b_enc2).reshape(-1, 1),
        Wd2=f32(W_dec2), bd2=f32(b_dec2).reshape(-1, 1),
        Wd1=f32(W_dec1), bd1=f32(b_dec1).reshape(-1, 1),
    )
    in_maps = [dict(com, xT=np.ascontiguousarray(xT[:, c * N:(c + 1) * N]))
               for c in range(n_cores)]
    res = run_bass_kernel_spmd(nc, in_maps, core_ids=list(range(n_cores)))
    global LAST_EXEC_NS, LAST_DBG
    LAST_EXEC_NS = res.exec_time_ns
    LAST_DBG = [res.results[c].get("dbg") for c in range(n_cores)]
    out = np.empty((cfg["n_tok"], cfg["d_model"]), np.float32)
    for c in range(n_cores):
        out[c * N:(c + 1) * N, :] = res.results[c]["recon"].T
    return out
